# revision 11
# baseline (speedup 1.0000x reference)
"""Self-contained TRN2 kernel for nn_FLASH_ShareA_FFConvM_FlashAttn.

kernel(**inputs) takes the full (unsharded) inputs from setup_inputs() and
returns the full (B, N, D) float32 output. Internally: data-parallel over the
batch - one batch sample per NeuronCore, 8 cores, no collectives.

v2: restructured for engine balance/overlap:
 - P0 LayerNorm in two passes with one batched Sqrt (no act-table thrash)
 - depthwise convs: identity folded into center tap; tiles split across
   PE (host-precomputed diag matrices), DVE (ts+add) and Act-assisted
 - P4 gating spread across all five engines; LN_o stats collected per-chunk
   but normalization deferred to one batched pass (sqrt once, scale via
   broadcast multiply + rank-1 matmul correction in P5)
 - zo^T kept SBUF-resident (no DRAM roundtrip); P5 reads it chunk-outer
"""
import sys

if "/opt/trn_rl_repo" not in sys.path:
    sys.path.insert(0, "/opt/trn_rl_repo")

import numpy as np
import ml_dtypes
import concourse.bass as bass
import concourse.bacc as bacc
import concourse.mybir as mybir
import concourse.tile as tile
from concourse import bass_utils

F32 = mybir.dt.float32
BF16 = mybir.dt.bfloat16
AF = mybir.ActivationFunctionType
OP = mybir.AluOpType

N, D, H, QK, G = 4096, 512, 2048, 128, 256
NG = N // G
NT = N // 128
KTAPS = 17
PAD = 8
E2 = 2 * D
EPS = 1e-5

# conv tile assignment: which channel tiles run on which engine
HID_PE = (2, 3, 4, 5, 6, 7, 8, 9)
HID_DVE = (0, 1, 10, 11, 12)
HID_ACT = (13, 14, 15)
O_PE = (0, 1)
QK_PE = True
# diag slot layout in the host-precomputed diags tensor
DIAG_SLOT = {("qk", 0): 0}
for _i, _hc in enumerate(HID_PE):
    DIAG_SLOT[("hid", _hc)] = 1 + _i
for _i, _oc in enumerate(O_PE):
    DIAG_SLOT[("o", _oc)] = 1 + len(HID_PE) + _i
NSLOTS = len(DIAG_SLOT)


def _conv_pe(nc, psum_pool, hpad, diag_sb, acc, evac):
    """acc = h + conv(h) on PE: 17 diag matmuls per 512-col chunk.
    diag_sb [128,17,128] has the +1 identity folded into tap 8."""
    for pair in range(4):
        ps = psum_pool.tile([128, 2, 512], F32, tag="convps")
        b0 = PAD + pair * 1024
        for k in range(KTAPS):
            s = k - PAD
            nc.tensor.matmul(ps[:, 0, :], diag_sb[:, k, :], hpad[:, b0 + s:b0 + s + 512],
                             start=(k == 0), stop=(k == KTAPS - 1))
            nc.tensor.matmul(ps[:, 1, :], diag_sb[:, k, :],
                             hpad[:, b0 + 512 + s:b0 + 1024 + s],
                             start=(k == 0), stop=(k == KTAPS - 1))
        evac.activation(acc[:, pair * 1024:(pair + 1) * 1024],
                        ps[:, :, :].rearrange("p a b -> p (a b)"), AF.Copy)


def _conv_dve(nc, pool, hpad, hpad1, dw_sb, dwi, acc):
    """acc = h + conv(h) on DVE: center tap (w8+1) as ts, 16x (ts + add)."""
    nc.vector.tensor_scalar(out=acc[:, :], in0=hpad[:, PAD:PAD + N],
                            scalar1=dw_sb[:, dwi, 8:9], scalar2=None, op0=OP.mult)
    for k in range(KTAPS):
        if k == 8:
            continue
        s = k - PAD
        if s % 2 == 0:
            src, off = hpad, PAD + s
        else:
            src, off = hpad1, PAD - 1 + s
        tmp = pool.tile([128, N], BF16, tag=f"cvs{k % 2}")
        nc.vector.tensor_scalar(out=tmp[:, :], in0=src[:, off:off + N],
                                scalar1=dw_sb[:, dwi, k:k + 1], scalar2=None, op0=OP.mult)
        nc.vector.tensor_add(acc[:, :], acc[:, :], tmp[:, :])


def _conv_act(nc, pool, hpad, dw_sb, dwi, acc):
    """acc = h + conv(h): tap products on Act (no alignment constraint),
    adds on DVE."""
    nc.vector.tensor_scalar(out=acc[:, :], in0=hpad[:, PAD:PAD + N],
                            scalar1=dw_sb[:, dwi, 8:9], scalar2=None, op0=OP.mult)
    for k in range(KTAPS):
        if k == 8:
            continue
        s = k - PAD
        tmp = pool.tile([128, N], BF16, tag=f"cvs{k % 2}")
        nc.scalar.activation(tmp[:, :], hpad[:, PAD + s:PAD + s + N], AF.Copy,
                             scale=dw_sb[:, dwi, k:k + 1])
        nc.vector.tensor_add(acc[:, :], acc[:, :], tmp[:, :])


def _emit(nc, tc, x, wh, wqk, wo, bh, bqk, bo, dwh, dwqk, dwo, gb, diags, wsum,
          out, spill, zspill):
    from contextlib import ExitStack
    es = ExitStack()
    consts = es.enter_context(tc.tile_pool(name="consts", bufs=1))
    wh_sb = consts.tile([128, 4, H], BF16)
    nc.sync.dma_start(wh_sb[:, :, :], wh.ap())
    wqk_sb = consts.tile([128, 4, QK], BF16)
    nc.sync.dma_start(wqk_sb[:, :, :], wqk.ap())
    wo_sb = consts.tile([128, 8, D], BF16)
    nc.sync.dma_start(wo_sb[:, :, :], wo.ap())
    bh_sb = consts.tile([128, 16], F32)
    nc.sync.dma_start(bh_sb[:, :], bh.ap())
    bqk_sb = consts.tile([128, 1], F32)
    nc.sync.dma_start(bqk_sb[:, :], bqk.ap())
    bo_sb = consts.tile([128, 4], F32)
    nc.sync.dma_start(bo_sb[:, :], bo.ap())
    dwh_sb = consts.tile([128, 16, KTAPS], F32)
    nc.sync.dma_start(dwh_sb[:, :, :], dwh.ap())
    dwqk_sb = consts.tile([128, 1, KTAPS], F32)
    nc.sync.dma_start(dwqk_sb[:, :, :], dwqk.ap())
    dwo_sb = consts.tile([128, 4, KTAPS], F32)
    nc.sync.dma_start(dwo_sb[:, :, :], dwo.ap())
    gb_sb = consts.tile([128, 8], F32)
    nc.sync.dma_start(gb_sb[:, :], gb.ap())
    wsum_sb = consts.tile([1, D], BF16)
    nc.sync.dma_start(wsum_sb[:, :], wsum.ap())
    eps_sb = consts.tile([128, 1], F32)
    nc.vector.memset(eps_sb[:, :], EPS)

    bigp = es.enter_context(tc.tile_pool(name="bigp", bufs=1))
    zT = bigp.tile([128, 4, N], BF16, tag="big4")

    # P0: token-shifted LayerNorm, two passes with one batched sqrt
    with tc.tile_pool(name="p0x", bufs=1) as p0x, \
         tc.tile_pool(name="p0s", bufs=2) as p0s:
        xbuf = p0x.tile([128, NT, D], F32)
        mvb = p0x.tile([128, NT, 2], F32)
        for tt in range(NT):
            t0 = tt * 128
            if tt == 0:
                nc.vector.memset(xbuf[0:1, 0, 0:D // 2], 0.0)
                nc.sync.dma_start(xbuf[1:128, 0, 0:D // 2], x[0:127, 0:D // 2])
            else:
                nc.sync.dma_start(xbuf[:, tt, 0:D // 2], x[t0 - 1:t0 + 127, 0:D // 2])
            nc.sync.dma_start(xbuf[:, tt, D // 2:D], x[t0:t0 + 128, D // 2:D])
            st6 = p0s.tile([128, 6], F32, tag="st6")
            nc.vector.bn_stats(st6[:, :], xbuf[:, tt, :])
            nc.vector.bn_aggr(mvb[:, tt, :], st6[:, :])
        rstd = p0x.tile([128, NT], F32)
        nc.scalar.activation(rstd[:, :], mvb[:, :, 1], AF.Sqrt, bias=eps_sb[:, :],
                             scale=1.0)
        nc.vector.reciprocal(rstd[:, :], rstd[:, :])
        nmu = p0x.tile([128, NT], F32)
        nc.vector.scalar_tensor_tensor(out=nmu[:, :], in0=mvb[:, :, 0], scalar=-1.0,
                                       in1=rstd[:, :], op0=OP.mult, op1=OP.mult)
        for tt in range(NT):
            zt = p0s.tile([128, D], BF16, tag="zt")
            nc.vector.tensor_scalar(out=zt[:, :], in0=xbuf[:, tt, :],
                                    scalar1=rstd[:, tt:tt + 1], scalar2=nmu[:, tt:tt + 1],
                                    op0=OP.mult, op1=OP.add)
            nc.sync.dma_start_transpose(zT[:, :, tt * 128:(tt + 1) * 128], zt[:, :])

    qsA = es.enter_context(tc.tile_pool(name="qsA", bufs=1))
    attnT = qsA.tile([128, NG, 2, G], BF16)
    lq_sb = qsA.tile([128, N], BF16)
    lk_str = qsA.tile([128, NT, 128], BF16)
    linkv_sb = qsA.tile([128, E2], BF16)
    linku_sb = qsA.tile([128, E2], BF16)

    # P1/P2: qk path + attention weights
    with tc.tile_pool(name="p1", bufs=1) as p1, \
         tc.tile_pool(name="p1s", bufs=2) as p1s, \
         tc.tile_pool(name="p1p", bufs=2, space="PSUM") as p1p, \
         tc.tile_pool(name="p1cp", bufs=1, space="PSUM") as p1cp:
        qkp = p1.tile([128, 2 * PAD + N], BF16, tag="qkpad")
        nc.vector.memset(qkp[:, 0:PAD], 0.0)
        nc.vector.memset(qkp[:, PAD + N:], 0.0)
        for ch in range(4):
            ps = p1p.tile([128, 2, 512], F32, tag="qkps")
            for kt in range(4):
                nc.tensor.matmul(ps[:, 0, :], wqk_sb[:, kt, :],
                                 zT[:, kt, ch * 1024:ch * 1024 + 512],
                                 start=(kt == 0), stop=(kt == 3))
                nc.tensor.matmul(ps[:, 1, :], wqk_sb[:, kt, :],
                                 zT[:, kt, ch * 1024 + 512:(ch + 1) * 1024],
                                 start=(kt == 0), stop=(kt == 3))
            nc.scalar.activation(qkp[:, PAD + ch * 1024:PAD + (ch + 1) * 1024],
                                 ps[:, :, :].rearrange("p a b -> p (a b)"),
                                 AF.Silu, bias=bqk_sb[:, :], scale=1.0)
        qkc = p1.tile([128, N], BF16, tag="qkc")
        if QK_PE:
            dqk = p1.tile([128, KTAPS, 128], BF16, tag="dqk")
            nc.sync.dma_start(dqk[:, :, :], diags.ap()[:, DIAG_SLOT[("qk", 0)], :, :])
            _conv_pe(nc, p1cp, qkp, dqk, qkc, nc.scalar)
        else:
            qkp1 = p1.tile([128, 2 * PAD + N], BF16, tag="qkpad1")
            nc.gpsimd.tensor_copy(qkp1[:, 0:2 * PAD + N - 2], qkp[:, 1:2 * PAD + N - 1])
            _conv_dve(nc, p1s, qkp, qkp1, dwqk_sb, 0, qkc)
        qq = p1.tile([128, N], BF16, tag="qq")
        qkk = p1.tile([128, N], BF16, tag="qkk")
        lkk = p1.tile([128, N], BF16, tag="lkk")
        for i, dst in ((0, qq), (1, lq_sb), (2, qkk), (3, lkk)):
            nc.vector.tensor_scalar(out=dst[:, :], in0=qkc[:, :], scalar1=gb_sb[:, i:i + 1],
                                    scalar2=gb_sb[:, 4 + i:5 + i], op0=OP.mult, op1=OP.add)
        nc.sync.dma_start_transpose(lk_str[:, :, :], lkk[:, :])

        for g in range(NG):
            for jh in range(2):
                sp = p1p.tile([128, G], F32, tag="simps")
                nc.tensor.matmul(sp[:, :], qkk[:, g * G + jh * 128: g * G + jh * 128 + 128],
                                 qq[:, g * G:(g + 1) * G], start=True, stop=True)
                rel = p1s.tile([128, G], BF16, tag="rel")
                nc.scalar.activation(rel[:, :], sp[:, :], AF.Relu)
                nc.vector.tensor_mul(attnT[:, g, jh, :], rel[:, :], rel[:, :])

    # P3: hidden path + conv + spill + lin_kv/lin_ku
    spill_v = spill.ap().rearrange("(tt p) (q c4) -> p tt q c4", p=128, c4=512)
    with tc.tile_pool(name="p3", bufs=2) as p3, \
         tc.tile_pool(name="p3q", bufs=1) as p3q, \
         tc.tile_pool(name="p3d", bufs=2) as p3d, \
         tc.tile_pool(name="p3a", bufs=1) as p3a, \
         tc.tile_pool(name="p3p", bufs=2, space="PSUM") as p3p, \
         tc.tile_pool(name="p3cp", bufs=1, space="PSUM") as p3cp, \
         tc.tile_pool(name="p3lp", bufs=1, space="PSUM") as p3lp:
        state = {"strips4": None}

        def produce(hc):
            hpad = p3.tile([128, 2 * PAD + N], BF16, tag="hpad")
            nc.vector.memset(hpad[:, 0:PAD], 0.0)
            nc.vector.memset(hpad[:, PAD + N:], 0.0)
            if hc in HID_PE:
                dg = p3d.tile([128, KTAPS, 128], BF16, tag="diag")
                nc.sync.dma_start(dg[:, :, :], diags.ap()[:, DIAG_SLOT[("hid", hc)], :, :])
            else:
                dg = None
            for cp in range(4):
                ps = p3p.tile([128, 2, 512], F32, tag="hps")
                for kt in range(4):
                    nc.tensor.matmul(ps[:, 0, :], wh_sb[:, kt, hc * 128:(hc + 1) * 128],
                                     zT[:, kt, cp * 1024:cp * 1024 + 512],
                                     start=(kt == 0), stop=(kt == 3))
                    nc.tensor.matmul(ps[:, 1, :], wh_sb[:, kt, hc * 128:(hc + 1) * 128],
                                     zT[:, kt, cp * 1024 + 512:(cp + 1) * 1024],
                                     start=(kt == 0), stop=(kt == 3))
                nc.scalar.activation(hpad[:, PAD + cp * 1024:PAD + (cp + 1) * 1024],
                                     ps[:, :, :].rearrange("p a b -> p (a b)"),
                                     AF.Silu, bias=bh_sb[:, hc:hc + 1], scale=1.0)
            return hpad, dg

        def convpost(hc, hpad, dg):
            if hc % 4 == 0:
                s4_new = p3q.tile([128, NT, 4, 128], BF16, tag="strips4")
                state["strips4"] = s4_new
            strips4 = state["strips4"]
            acc = p3.tile([128, N], BF16, tag="acc")
            if hc in HID_PE:
                _conv_pe(nc, p3cp, hpad, dg, acc, nc.scalar)
            elif hc in HID_DVE:
                hpad1 = p3q.tile([128, 2 * PAD + N], BF16, tag="hpad1")
                nc.gpsimd.tensor_copy(hpad1[:, 0:2 * PAD + N - 2], hpad[:, 1:2 * PAD + N - 1])
                _conv_dve(nc, p3a, hpad, hpad1, dwh_sb, hc, acc)
            else:
                _conv_act(nc, p3a, hpad, dwh_sb, hc, acc)
            nc.sync.dma_start_transpose(strips4[:, :, hc % 4, :], acc[:, :])
            if hc % 4 == 3:
                q = hc // 4
                nc.sync.dma_start(spill_v[:, :, q, :], strips4[:, :, :, :])
                lp = p3lp.tile([128, 512], F32, tag="linps")
                for tt in range(NT):
                    nc.tensor.matmul(
                        lp[:, :], lk_str[:, tt, :],
                        strips4[:, tt, :, :].rearrange("p a c -> p (a c)"),
                        start=(tt == 0), stop=(tt == NT - 1))
                dst = linkv_sb if hc < 8 else linku_sb
                nc.scalar.activation(dst[:, (q % 2) * 512:(q % 2) * 512 + 512],
                                     lp[:, :], AF.Copy)

        prev = None
        for hc in range(16):
            pr = produce(hc)
            if prev is not None:
                convpost(*prev)
            prev = (hc, *pr)
        convpost(*prev)

    p4e = es.enter_context(tc.tile_pool(name="p4e", bufs=1))
    mv_all = p4e.tile([128, NT, 2], F32)
    zsp_v = zspill.ap().rearrange("a p t -> p a t")

    # P4: attention + gating; LN_o stats collected, normalization deferred
    with tc.tile_pool(name="p4", bufs=2) as p4, \
         tc.tile_pool(name="p4p", bufs=2, space="PSUM") as p4p:
        for g in range(NG):
            vg, ug = [], []
            for jh in range(2):
                vt = p4.tile([128, E2], BF16, tag=f"vg{jh}")
                nc.sync.dma_start(vt[:, :], spill[g * G + jh * 128: g * G + jh * 128 + 128, 0:E2])
                ut = p4.tile([128, E2], BF16, tag=f"ug{jh}")
                nc.sync.dma_start(ut[:, :], spill[g * G + jh * 128: g * G + jh * 128 + 128, E2:H])
                vg.append(vt)
                ug.append(ut)
            for it in range(2):
                tti = g * 2 + it
                islice = slice(g * G + it * 128, g * G + it * 128 + 128)
                apv = p4p.tile([128, E2], F32, tag="apv")
                apu = p4p.tile([128, E2], F32, tag="apu")
                for ap_, grp, lin in ((apv, vg, linkv_sb), (apu, ug, linku_sb)):
                    for e in range(2):
                        for jh in range(2):
                            nc.tensor.matmul(ap_[:, e * 512:(e + 1) * 512],
                                             attnT[:, g, jh, it * 128:it * 128 + 128],
                                             grp[jh][:, e * 512:(e + 1) * 512],
                                             start=(jh == 0), stop=False)
                        nc.tensor.matmul(ap_[:, e * 512:(e + 1) * 512],
                                         lq_sb[:, islice], lin[:, e * 512:(e + 1) * 512],
                                         start=False, stop=True)
                t1 = p4.tile([128, E2], BF16, tag="t1")
                nc.vector.tensor_mul(t1[:, :], ug[it][:, :], apv[:, :])
                sg = p4.tile([128, E2], BF16, tag="sg")
                nc.scalar.activation(sg[:, :], t1[:, :], AF.Sigmoid)
                aus = p4.tile([128, E2], BF16, tag="aus")
                nc.scalar.activation(aus[:, :], apu[:, :], AF.Copy)
                t2 = p4.tile([128, E2], BF16, tag="t2")
                nc.gpsimd.tensor_mul(t2[:, :], vg[it][:, :], aus[:, :])
                go = p4.tile([128, 2, 512], BF16, tag="go")
                gof = go[:, :, :].rearrange("p a b -> p (a b)")
                nc.vector.tensor_mul(gof, t2[:, :], sg[:, :])
                st12 = p4.tile([128, 2, 6], F32, tag="st12")
                nc.vector.bn_stats(st12[:, 0, :], go[:, 0, :])
                nc.vector.bn_stats(st12[:, 1, :], go[:, 1, :])
                nc.vector.bn_aggr(mv_all[:, tti, :], st12[:, :, :])
                zot = p4.tile([128, 8, 128], BF16, tag="zot")
                nc.sync.dma_start_transpose(zot[:, :, :], gof)
                nc.sync.dma_start(zsp_v[:, :, tti * 128:(tti + 1) * 128], zot[:, :, :])

    # P4 epilogue: batched LN_o scale factors (one sqrt; transpose the
    # per-token (rstd, -mu*rstd) pairs into free-major layout)
    rstdp = p4e.tile([128, NT], F32)
    nc.scalar.activation(rstdp[:, :], mv_all[:, :, 1], AF.Sqrt, bias=eps_sb[:, :],
                         scale=1.0)
    nc.vector.reciprocal(rstdp[:, :], rstdp[:, :])
    cmb8 = p4e.tile([128, 4, NT], BF16)
    nc.vector.tensor_copy(cmb8[:, 0, :], rstdp[:, :])
    nc.vector.scalar_tensor_tensor(out=cmb8[:, 1, :], in0=mv_all[:, :, 0], scalar=-1.0,
                                   in1=rstdp[:, :], op0=OP.mult, op1=OP.mult)
    nc.vector.memset(cmb8[:, 2:4, :], 0.0)
    cmbT = p4e.tile([4 * NT, 128], BF16)
    nc.sync.dma_start_transpose(cmbT[:, :], cmb8[:, :, :].rearrange("p a b -> p (a b)"))
    rstd_f8 = p4e.tile([1, NT, 128], BF16)
    nc.sync.dma_start(rstd_f8[0:1, :, :], cmbT[0:NT, :])
    nmu_f8 = p4e.tile([1, NT, 128], BF16)
    nc.sync.dma_start(nmu_f8[0:1, :, :], cmbT[NT:2 * NT, :])
    nmu_f8 = nmu_f8[:, :, :].rearrange("p a b -> p (a b)")
    rstd_bc = p4e.tile([128, N], BF16)
    nc.gpsimd.partition_broadcast(rstd_bc[:, :], rstd_f8[:, :, :].rearrange("p a b -> p (a b)"))

    # P5: output FFConvM, chunk-outer (zoT read once), rank-1 nmu correction
    vo_big = bigp.tile([128, NT, 4, 128], BF16, tag="big4")
    with tc.tile_pool(name="p5", bufs=1) as p5, \
         tc.tile_pool(name="p5s", bufs=1) as p5s, \
         tc.tile_pool(name="p5z", bufs=1) as p5z, \
         tc.tile_pool(name="p5d", bufs=2) as p5d, \
         tc.tile_pool(name="p5p", bufs=1, space="PSUM") as p5p, \
         tc.tile_pool(name="p5cp", bufs=1, space="PSUM") as p5cp:
        hpads = []
        for oc in range(4):
            hp = p5.tile([128, 2 * PAD + N], BF16, tag=f"hpad5{oc}")
            nc.vector.memset(hp[:, 0:PAD], 0.0)
            nc.vector.memset(hp[:, PAD + N:], 0.0)
            hpads.append(hp)
        for ck in range(8):
            ps4 = p5p.tile([128, 4, 512], F32, tag="ops")
            c0 = ck * 512
            zoc = p5z.tile([128, 8, 512], BF16, tag="zoc")
            nc.sync.dma_start(zoc[:, :, :], zsp_v[:, :, c0:c0 + 512])
            for kt in range(8):
                nc.vector.tensor_mul(zoc[:, kt, :], zoc[:, kt, :], rstd_bc[:, c0:c0 + 512])
            for kt in range(8):
                for oc in range(4):
                    nc.tensor.matmul(ps4[:, oc, :], wo_sb[:, kt, oc * 128:(oc + 1) * 128],
                                     zoc[:, kt, :], start=(kt == 0), stop=False)
            for oc in range(4):
                nc.tensor.matmul(ps4[:, oc, :], wsum_sb[0:1, oc * 128:(oc + 1) * 128],
                                 nmu_f8[0:1, c0:c0 + 512], start=False, stop=True)
                nc.scalar.activation(hpads[oc][:, PAD + c0:PAD + c0 + 512], ps4[:, oc, :],
                                     AF.Silu, bias=bo_sb[:, oc:oc + 1], scale=1.0)

        for oc in range(4):
            acc = p5s.tile([128, N], BF16, tag="acc5")
            if oc in O_PE:
                dg = p5d.tile([128, KTAPS, 128], BF16, tag="diag5")
                nc.sync.dma_start(dg[:, :, :], diags.ap()[:, DIAG_SLOT[("o", oc)], :, :])
                _conv_pe(nc, p5cp, hpads[oc], dg, acc, nc.scalar)
            else:
                hpad1 = p5.tile([128, 2 * PAD + N], BF16, tag="hpad51")
                nc.gpsimd.tensor_copy(hpad1[:, 0:2 * PAD + N - 2],
                                      hpads[oc][:, 1:2 * PAD + N - 1])
                _conv_dve(nc, p5s, hpads[oc], hpad1, dwo_sb, oc, acc)
            nc.sync.dma_start_transpose(vo_big[:, :, oc, :], acc[:, :])

    # P6: residual
    with tc.tile_pool(name="p6", bufs=3) as p6:
        for tt in range(NT):
            xt = p6.tile([128, D], F32, tag="xt6")
            nc.sync.dma_start(xt[:, :], x[tt * 128:(tt + 1) * 128, :])
            of = p6.tile([128, D], F32, tag="of")
            nc.gpsimd.tensor_add(of[:, :], xt[:, :],
                                 vo_big[:, tt, :, :].rearrange("p a c -> p (a c)"))
            nc.sync.dma_start(out[tt * 128:(tt + 1) * 128, :], of[:, :])
    es.close()


def _build_nc():
    nc = bacc.Bacc("TRN2", target_bir_lowering=False, debug=False)
    x = nc.dram_tensor("x", [N, D], F32, kind="ExternalInput")
    wh = nc.dram_tensor("wh", [128, 4, H], BF16, kind="ExternalInput")
    wqk = nc.dram_tensor("wqk", [128, 4, QK], BF16, kind="ExternalInput")
    wo = nc.dram_tensor("wo", [128, 8, D], BF16, kind="ExternalInput")
    bh = nc.dram_tensor("bh", [128, 16], F32, kind="ExternalInput")
    bqk = nc.dram_tensor("bqk", [128, 1], F32, kind="ExternalInput")
    bo = nc.dram_tensor("bo", [128, 4], F32, kind="ExternalInput")
    dwh = nc.dram_tensor("dwh", [128, 16, KTAPS], F32, kind="ExternalInput")
    dwqk = nc.dram_tensor("dwqk", [128, 1, KTAPS], F32, kind="ExternalInput")
    dwo = nc.dram_tensor("dwo", [128, 4, KTAPS], F32, kind="ExternalInput")
    gb = nc.dram_tensor("gb", [128, 8], F32, kind="ExternalInput")
    diags = nc.dram_tensor("diags", [128, NSLOTS, KTAPS, 128], BF16, kind="ExternalInput")
    wsum = nc.dram_tensor("wsum", [1, D], BF16, kind="ExternalInput")
    out = nc.dram_tensor("out", [N, D], F32, kind="ExternalOutput")
    spill = nc.dram_tensor("spill", [N, H], BF16)
    zspill = nc.dram_tensor("zspill", [8, 128, N], BF16)
    with tile.TileContext(nc) as tc:
        _emit(nc, tc, x, wh, wqk, wo, bh, bqk, bo, dwh, dwqk, dwo, gb, diags, wsum,
              out, spill, zspill)
    nc.compile()
    return nc


def prep_inputs(inputs):
    f32 = np.float32
    bf = ml_dtypes.bfloat16
    W_h = np.asarray(inputs["W_h"], f32)
    W_qk = np.asarray(inputs["W_qk"], f32)
    W_o = np.asarray(inputs["W_o"], f32)
    whp = np.asarray(inputs["ln_h_g"], f32)[:, None] * W_h
    bhp = np.asarray(inputs["ln_h_b"], f32) @ W_h + np.asarray(inputs["b_h"], f32)
    wqkp = np.asarray(inputs["ln_qk_g"], f32)[:, None] * W_qk
    bqkp = np.asarray(inputs["ln_qk_b"], f32) @ W_qk + np.asarray(inputs["b_qk"], f32)
    wop = np.asarray(inputs["ln_o_g"], f32)[:, None] * W_o
    bop = np.asarray(inputs["ln_o_b"], f32) @ W_o + np.asarray(inputs["b_o"], f32)
    gamma = np.asarray(inputs["gamma"], f32).copy()
    beta = np.asarray(inputs["beta"], f32).copy()
    gamma[0] /= G
    beta[0] /= G
    gamma[3] /= N
    beta[3] /= N

    # fold the residual identity into the center conv tap
    dw_h = np.asarray(inputs["dw_h"], f32).copy()
    dw_qk = np.asarray(inputs["dw_qk"], f32).copy()
    dw_o = np.asarray(inputs["dw_o"], f32).copy()
    for dw in (dw_h, dw_qk, dw_o):
        dw[8, :] += 1.0

    def lhsT(w, ktiles):
        return np.ascontiguousarray(w.reshape(ktiles, 128, -1).transpose(1, 0, 2)).astype(bf)

    def chan(v, ntiles):
        return np.ascontiguousarray(v.reshape(ntiles, 128).T).astype(f32)

    def dwl(dw, ntiles):
        return np.ascontiguousarray(
            dw.T.reshape(ntiles, 128, KTAPS).transpose(1, 0, 2)).astype(f32)

    # host-precomputed diag matrices for the PE convs
    diags = np.zeros((128, NSLOTS, KTAPS, 128), f32)
    ar = np.arange(128)
    for (kind, idx), slot in DIAG_SLOT.items():
        dw = {"qk": dw_qk, "hid": dw_h, "o": dw_o}[kind]
        wt = dw[:, idx * 128:(idx + 1) * 128]  # [K, 128]
        for k in range(KTAPS):
            diags[ar, slot, k, ar] = wt[k]

    return {
        "wh": lhsT(whp, 4), "wqk": lhsT(wqkp, 4), "wo": lhsT(wop, 8),
        "bh": chan(bhp, 16), "bqk": chan(bqkp, 1), "bo": chan(bop, 4),
        "dwh": dwl(dw_h, 16), "dwqk": dwl(dw_qk, 1), "dwo": dwl(dw_o, 4),
        "gb": np.concatenate([gamma.T, beta.T], axis=1).astype(f32),
        "diags": diags.astype(bf),
        "wsum": np.ascontiguousarray(wop.sum(axis=0)[None, :]).astype(bf),
    }


_NC = None


def get_nc():
    global _NC
    if _NC is None:
        _NC = _build_nc()
    return _NC


def make_in_maps(inputs):
    x = np.asarray(inputs["x"], np.float32)
    B = x.shape[0]
    prep = prep_inputs(inputs)
    return [{"x": np.ascontiguousarray(x[b]), **prep} for b in range(B)]


def kernel(**inputs):
    nc = get_nc()
    in_maps = make_in_maps(inputs)
    res = bass_utils.run_bass_kernel_spmd(nc, in_maps, core_ids=list(range(8)))
    out = np.stack([res.results[b]["out"] for b in range(8)], axis=0)
    return out.astype(np.float32)
